# revision 53
# baseline (speedup 1.0000x reference)
"""Disordered causal self-attention for Trainium2 (Bass/Tile), 8 NeuronCores.

Reference computation (B=4, T=2048, C=1024, NH=16, D=64):
    qkv = x @ W_attn + b_attn ; split q,k,v ; q += bQ[h], k += bK[h]
    att = softmax(causal(q k^T / sqrt(D))) ; y = att @ v
    out = y @ W_proj + b_proj

Sharding: 8 cores = 4 batches x 2 head-groups of 8 heads. Each core computes
its batch's QKV projection for its 8 heads, the attention, and a partial
output projection (its heads' rows of W_proj). Host sums the two head-group
partials and adds the bias terms.

Device-side formulation (all matmuls bf16 -> fp32 PSUM):
  - x is fed pre-transposed as xT [C, T], so Q/K projections produce
    qT/kT in [d, t] layout directly (per-head-pair 128-partition tiles,
    even head in partitions 0:64, odd head in 64:128).
  - Scores are computed transposed: S^T[tk, tq] = kT^T-chunk . qT, with the
    even/odd heads of a pair packed onto PE row-groups (0,0)/(64,0) since
    the contraction dim is only D=64.
  - exp() runs on ScalarE straight out of PSUM with scale=1/8; no max
    subtraction (score range is tiny for this input distribution).
  - Causal masking multiplies the diagonal-overlap blocks by a precomputed
    0/1 strip; fully-invalid chunks are never computed.
  - V is produced in [t, d] layout with a ones column appended, so the
    y^T accumulation y^T[d, tq] += v_aug^T . E^T also yields the softmax
    denominator in row 64 of each PSUM tile.
  - Normalization: both heads' denominator rows are packed into partition 0,
    scattered across partitions with a 32x32 DVE stream transpose so the
    (8-cycle/element, per-lane) reciprocal runs 32 lanes wide, transposed
    back, broadcast over the partition dim with two accumulating K=1
    outer-product matmuls, then one DVE multiply per head. The V projection
    bias is folded on the host (softmax rows sum to 1 => y += b_v exactly),
    as is b_proj.
  - Emission is a flat software pipeline over (pair, tq-block, tk-chunk)
    items: S/exp of item i+1 are emitted before the y-matmuls of item i,
    and the q/k/v projections of later pairs plus the normalize chains and
    (for the last pair) output-projection tiles are drip-fed between chunks
    so the in-order PE queue always has independent work while ScalarE
    streams the exps.
"""

import sys

sys.path.insert(0, "/opt/trn_rl_repo")

import numpy as np
import ml_dtypes

import concourse.bacc as bacc
import concourse.bass as bass
import concourse.mybir as mybir
import concourse.tile as tile

BF16 = mybir.dt.bfloat16
F32 = mybir.dt.float32
NPBF16 = ml_dtypes.bfloat16

B, T, C = 4, 2048, 1024
NH, D = 16, 64
HC = NH // 2  # heads per core (head-group split across 2 cores per batch)
J = HC // 2  # head pairs per core
N_CORES = 8


def build_nc(t=T, c=C, j_pairs=J):
    """Build the per-core Bass program. All 8 cores run this same program."""
    ck = c // 128  # contraction chunks for the input projections
    tqb = t // 512  # tq blocks
    tkc = t // 128  # tk chunks
    hc = 2 * j_pairs

    nc = bacc.Bacc("TRN2", target_bir_lowering=False, debug=False, num_devices=N_CORES)

    xT = nc.dram_tensor("xT", [c, t], BF16, kind="ExternalInput")
    wq = nc.dram_tensor("wq", [c, 128 * j_pairs], BF16, kind="ExternalInput")
    wk = nc.dram_tensor("wk", [c, 128 * j_pairs], BF16, kind="ExternalInput")
    wv = nc.dram_tensor("wv", [c, D * hc], BF16, kind="ExternalInput")
    wp = nc.dram_tensor("wp", [128 * j_pairs, c], BF16, kind="ExternalInput")
    qb = nc.dram_tensor("qb", [128, j_pairs], F32, kind="ExternalInput")
    kb = nc.dram_tensor("kb", [128, j_pairs], F32, kind="ExternalInput")
    strip = nc.dram_tensor("strip", [128, 512], BF16, kind="ExternalInput")
    out = nc.dram_tensor("out", [t, c], F32, kind="ExternalOutput")

    with tile.TileContext(nc) as tc:
        with (
            tc.tile_pool(name="weights", bufs=1) as wpool,
            tc.tile_pool(name="acts", bufs=1) as apool,
            tc.tile_pool(name="pe_sb", bufs=7) as epool,
            tc.tile_pool(name="pr_sb", bufs=3) as rpool,
            tc.tile_pool(name="pout_sb", bufs=3) as opool,
            tc.tile_pool(name="pp_mm", bufs=2, space="PSUM") as pmm,
            tc.tile_pool(name="pp_s", bufs=2, space="PSUM") as ps,
            tc.tile_pool(name="pp_y", bufs=1, space="PSUM") as py,
        ):
            # ---- load inputs to SBUF ----
            # per-chunk tiles so the first projection matmuls only wait for
            # the chunks they contract over, not the whole input DMA
            xT_sb = [wpool.tile([128, t], BF16, name=f"xT{cc}") for cc in range(ck)]
            wq_sb = [wpool.tile([128, 128 * j_pairs], BF16, name=f"wq{cc}") for cc in range(ck)]
            wk_sb = [wpool.tile([128, 128 * j_pairs], BF16, name=f"wk{cc}") for cc in range(ck)]
            wv_sb = [wpool.tile([128, D * hc], BF16, name=f"wv{cc}") for cc in range(ck)]
            for cc in range(ck):
                eng = (nc.gpsimd, nc.sync, nc.scalar)[cc % 3]
                eng.dma_start(out=xT_sb[cc][:], in_=xT[cc * 128 : (cc + 1) * 128, :])
                nc.sync.dma_start(out=wq_sb[cc][:], in_=wq[cc * 128 : (cc + 1) * 128, :])
                nc.scalar.dma_start(out=wk_sb[cc][:], in_=wk[cc * 128 : (cc + 1) * 128, :])
                nc.gpsimd.dma_start(out=wv_sb[cc][:], in_=wv[cc * 128 : (cc + 1) * 128, :])
            wp_sb = wpool.tile([128, j_pairs, c], BF16)
            for jj in range(j_pairs):
                nc.scalar.dma_start(out=wp_sb[:, jj, :], in_=wp[jj * 128 : (jj + 1) * 128, :])
            qb_sb = wpool.tile([128, j_pairs], F32)
            nc.sync.dma_start(out=qb_sb[:], in_=qb[:])
            kb_sb = wpool.tile([128, j_pairs], F32)
            nc.sync.dma_start(out=kb_sb[:], in_=kb[:])
            strip_sb = wpool.tile([128, 512], BF16)
            nc.sync.dma_start(out=strip_sb[:], in_=strip[:])

            # selectors for the denominator broadcast: two accumulating K=1
            # outer products map head A's reciprocals to out partitions 0:64
            # and head B's to 64:128
            sel_sb = wpool.tile([1, 256], BF16)
            nc.vector.memset(sel_sb[:], 0.0)
            nc.vector.memset(sel_sb[0:1, 0:64], 1.0)
            nc.vector.memset(sel_sb[0:1, 192:256], 1.0)
            selA = sel_sb[0:1, 0:128]
            selB = sel_sb[0:1, 128:256]

            # per-head-pair activation tensors (separate tiles so the Tile
            # scheduler can start attention on pair j before later pairs'
            # projections finish)
            qT_sb = [apool.tile([128, t], BF16, name=f"qT{jj}") for jj in range(j_pairs)]
            kT_sb = [apool.tile([128, t], BF16, name=f"kT{jj}") for jj in range(j_pairs)]
            yT_sb = {
                (jj, qq): apool.tile([128, 512], BF16, name=f"yT{jj}_{qq}")
                for jj in range(j_pairs)
                for qq in range(tqb)
            }
            # v in [t, d] layout, per tk-chunk tiles, with a ones column (d=D)
            v_sb = [apool.tile([128, hc, D + 1], BF16, name=f"v{tt}") for tt in range(tkc)]

            def qk_tile_steps(jj, tb):
                pq = pmm.tile([128, 512], F32, tag="mm", name=f"pq{jj}_{tb}")
                pk = pmm.tile([128, 512], F32, tag="mm", name=f"pk{jj}_{tb}")

                def half(p, w_sb, lo_cc, hi_cc):
                    for cc in range(lo_cc, hi_cc):
                        nc.tensor.matmul(
                            p[:],
                            lhsT=w_sb[cc][:, jj * 128 : (jj + 1) * 128],
                            rhs=xT_sb[cc][:, tb * 512 : (tb + 1) * 512],
                            start=(cc == 0),
                            stop=(cc == ck - 1),
                        )

                yield lambda: half(pq, wq_sb, 0, ck // 2)
                yield lambda: half(pq, wq_sb, ck // 2, ck)
                yield lambda: nc.vector.tensor_scalar_add(
                    qT_sb[jj][:, tb * 512 : (tb + 1) * 512], pq[:], qb_sb[:, jj : jj + 1]
                )
                yield lambda: half(pk, wk_sb, 0, ck // 2)
                yield lambda: half(pk, wk_sb, ck // 2, ck)
                yield lambda: nc.vector.tensor_scalar_add(
                    kT_sb[jj][:, tb * 512 : (tb + 1) * 512], pk[:], kb_sb[:, jj : jj + 1]
                )

            def qk_tile(jj, tb):
                for step in qk_tile_steps(jj, tb):
                    step()

            def v_tile_steps(tt):
                pv = pmm.tile([128, D * hc], F32, tag="mm", name=f"pv{tt}")

                def half(lo_cc, hi_cc):
                    for cc in range(lo_cc, hi_cc):
                        nc.tensor.matmul(
                            pv[:],
                            lhsT=xT_sb[cc][:, tt * 128 : (tt + 1) * 128],
                            rhs=wv_sb[cc][:],
                            start=(cc == 0),
                            stop=(cc == ck - 1),
                        )

                def evict():
                    nc.vector.tensor_copy(
                        out=v_sb[tt][:, :, 0:D],
                        in_=pv[:].rearrange("p (h d) -> p h d", h=hc),
                    )
                    nc.vector.memset(v_sb[tt][:, :, D : D + 1], 1.0)

                yield lambda: half(0, ck // 2)
                yield lambda: half(ck // 2, ck)
                yield evict

            def v_tile(tt):
                for step in v_tile_steps(tt):
                    step()

            # ---- flat software-pipelined attention stream ----
            # Items are (pair, tq-block, tk-chunk). The S-matmuls + exp of
            # item i are emitted BEFORE the y-matmuls of item i-1, so the
            # in-order PE queue is never parked on the exp it is about to
            # consume. Projection tiles and normalize chains are drip-fed
            # as fillers between chunks to fill PE slack in the (ScalarE
            # bound) attention stream.
            y_tiles = {}
            fillers = []

            def chunk_S(jj, qblk, tt):
                off = max(0, tt * 128 - qblk * 512)
                s_ps = ps.tile([128, 1024], F32, tag="s", name=f"s{jj}_{qblk}_{tt}")
                for h in range(2):
                    nc.tensor.matmul(
                        s_ps[:, h * 512 + off : (h + 1) * 512],
                        lhsT=kT_sb[jj][h * 64 : (h + 1) * 64, tt * 128 : (tt + 1) * 128],
                        rhs=qT_sb[jj][
                            h * 64 : (h + 1) * 64, qblk * 512 + off : (qblk + 1) * 512
                        ],
                        start=True,
                        stop=True,
                        tile_position=(h * 64, 0),
                    )
                e_sb = epool.tile([128, 1024], BF16, tag="e", name=f"e{jj}_{qblk}_{tt}")
                if off == 0:
                    nc.scalar.activation(
                        e_sb[:], s_ps[:], mybir.ActivationFunctionType.Exp, scale=0.125
                    )
                else:
                    for h in range(2):
                        lo = h * 512
                        nc.scalar.activation(
                            e_sb[:, lo + off : lo + 512],
                            s_ps[:, lo + off : lo + 512],
                            mybir.ActivationFunctionType.Exp,
                            scale=0.125,
                        )
                if tt * 128 >= qblk * 512:
                    for h in range(2):
                        lo = h * 512
                        nc.vector.tensor_mul(
                            e_sb[:, lo + off : lo + off + 128],
                            e_sb[:, lo + off : lo + off + 128],
                            strip_sb[:, 384:512],
                        )
                return e_sb

            def make_normalize(jj, qblk, y_sbs):
                def normalize():
                    # batched reciprocal of both heads' denominator rows:
                    # pack them side by side in partition 0, scatter across
                    # partitions with a 32x32 stream transpose, reciprocate
                    # at 32 lanes, transpose back, then broadcast over the
                    # partition dim with two accumulating K=1 outer products
                    scr = rpool.tile([32, 1024], F32, tag="scr", bufs=3, name=f"sc{jj}_{qblk}")
                    nc.vector.memset(scr[:], 1.0)
                    nc.vector.tensor_copy(out=scr[0:1, 0:512], in_=y_sbs[0][64:65, :])
                    nc.vector.tensor_copy(out=scr[0:1, 512:1024], in_=y_sbs[1][64:65, :])
                    tr = rpool.tile([32, 1024], F32, tag="tr", bufs=3, name=f"tr{jj}_{qblk}")
                    nc.vector.transpose(out=tr[:], in_=scr[:])
                    trr = rpool.tile([32, 1024], BF16, tag="trr", bufs=3, name=f"tq{jj}_{qblk}")
                    nc.vector.memset(trr[:], 1.0)
                    v_tr = tr[:].rearrange("p (k b) -> p k b", b=32)[:, :, 0:1]
                    v_trr = trr[:].rearrange("p (k b) -> p k b", b=32)[:, :, 0:1]
                    with nc.allow_low_precision(reason="bf16 denom broadcast"):
                        nc.vector.reciprocal(v_trr, v_tr)
                    rrow = rpool.tile([32, 1024], BF16, tag="rrow", bufs=3, name=f"rw{jj}_{qblk}")
                    nc.vector.transpose(out=rrow[:], in_=trr[:])
                    rb_ps = pmm.tile([128, 512], F32, tag="mm", name=f"rb{jj}_{qblk}")
                    nc.tensor.matmul(
                        rb_ps[:], lhsT=selA, rhs=rrow[0:1, 0:512], start=True, stop=False
                    )
                    nc.tensor.matmul(
                        rb_ps[:], lhsT=selB, rhs=rrow[0:1, 512:1024], start=False, stop=True
                    )
                    for h in range(2):
                        nc.vector.tensor_mul(
                            yT_sb[(jj, qblk)][h * 64 : (h + 1) * 64, :],
                            y_sbs[h][0:64, :],
                            rb_ps[h * 64 : (h + 1) * 64, :],
                        )
                    if jj == j_pairs - 1:
                        for tt in range(4 * qblk, 4 * qblk + 4):
                            fillers.append(lambda tt=tt: proj_tile(tt))

                return normalize

            def chunk_y(jj, qblk, tt, e_sb):
                n_tk = 4 * (qblk + 1)
                off = max(0, tt * 128 - qblk * 512)
                yp = y_tiles[(jj, qblk)]
                for h in range(2):
                    nc.tensor.matmul(
                        yp[h][:, off:512],
                        lhsT=v_sb[tt][:, 2 * jj + h, :],
                        rhs=e_sb[:, h * 512 + off : (h + 1) * 512],
                        start=(tt == 0),
                        stop=(tt == n_tk - 1),
                    )
                if tt == n_tk - 1:
                    # evict y (+denominator row) so the PSUM slot frees fast;
                    # the reciprocal/broadcast chain runs later as a filler
                    y_sbs = []
                    for h in range(2):
                        y_sb = rpool.tile(
                            [65, 512], F32, tag="ysb", bufs=6, name=f"ysb{jj}_{qblk}_{h}"
                        )
                        nc.vector.tensor_copy(out=y_sb[:], in_=yp[h][:])
                        y_sbs.append(y_sb)
                    fillers.append(make_normalize(jj, qblk, y_sbs))

            projected = set()

            def proj_tile(tt):
                projected.add(tt)
                for cb in range(max(1, c // 512)):
                    cw = min(512, c)
                    po = pmm.tile([128, cw], F32, tag="mm", name=f"po{tt}_{cb}")
                    for jj in range(j_pairs):
                        nc.tensor.matmul(
                            po[:],
                            lhsT=yT_sb[(jj, tt // 4)][:, (tt % 4) * 128 : (tt % 4 + 1) * 128],
                            rhs=wp_sb[:, jj, cb * cw : (cb + 1) * cw],
                            start=(jj == 0),
                            stop=(jj == j_pairs - 1),
                        )
                    o_sb = opool.tile([128, cw], F32, tag="o")
                    nc.vector.tensor_copy(out=o_sb[:], in_=po[:])
                    nc.sync.dma_start(
                        out=out[tt * 128 : (tt + 1) * 128, cb * cw : (cb + 1) * cw],
                        in_=o_sb[:],
                    )

            # lead-in: q/k for pair 0 and the first v chunks
            for tb in range(tqb):
                qk_tile(0, tb)
            for tt in range(min(4, tkc)):
                v_tile(tt)

            items = [
                (jj, qblk, tt, qblk == 0)
                for jj in range(j_pairs)
                for qblk in range(tqb)
                for tt in range(4 * (qblk + 1))
            ]
            filler_gens = []
            v_emitted = [4]  # v tiles fully emitted (lead-in does 0..3)
            qk_emitted = [4, 0, 0, 0]

            def pump(n=1):
                # emit one whole filler tile (half-tile pacing measured worse)
                if not filler_gens:
                    return
                gen, (kind, idx) = filler_gens.pop(0)
                for step in gen:
                    step()
                if kind == "v":
                    v_emitted[0] = idx + 1
                elif kind == "qk":
                    qk_emitted[idx] += 1

            def drain_until(cond):
                while not cond() and filler_gens:
                    pump()
                assert cond()

            pending_y = []
            for jj, qblk, tt, first_blk in items:
                if tt == 0:
                    y_tiles[(jj, qblk)] = [
                        py.tile([65, 512], F32, tag="yA", name=f"yA{jj}_{qblk}"),
                        py.tile([65, 512], F32, tag="yB", name=f"yB{jj}_{qblk}"),
                    ]
                    if jj == 0:
                        for vt in range(4 * (qblk + 1), min(4 * (qblk + 2), tkc)):
                            filler_gens.append((v_tile_steps(vt), ("v", vt)))
                    if jj + 1 < j_pairs:
                        filler_gens.append((qk_tile_steps(jj + 1, qblk), ("qk", jj + 1)))
                    if first_blk and jj > 0:
                        drain_until(lambda: qk_emitted[jj] >= tqb)
                if jj == 0:
                    drain_until(lambda: v_emitted[0] > tt)
                e_sb = chunk_S(jj, qblk, tt)
                pending_y.append((jj, qblk, tt, e_sb))
                if len(pending_y) > 1:
                    chunk_y(*pending_y.pop(0))
                if tt % 2 == 1 or len(filler_gens) > 3:
                    pump()
                elif fillers:
                    fillers.pop(0)()
            while pending_y:
                chunk_y(*pending_y.pop(0))
            while filler_gens:
                pump()
            while fillers:
                fillers.pop(0)()
            for tt in range(tkc):
                if tt not in projected:
                    proj_tile(tt)

    nc.compile()
    return nc


_NC_CACHE = {}
TRACE = False  # test harness sets this to capture an NTFF profile
LAST_RES = None


def _get_nc():
    if "nc" not in _NC_CACHE:
        _NC_CACHE["nc"] = build_nc()
    return _NC_CACHE["nc"]


def _prep_core_inputs(x, W_attn, b_attn, bQ, bK, W_proj, strip_np, b_, hg):
    heads = [hg * HC + h for h in range(HC)]
    # column order for the q/k weight tiles: pair j -> heads (2j, 2j+1)
    cols = np.concatenate([np.arange(h * D, (h + 1) * D) for h in heads])
    xT = np.ascontiguousarray(x[b_].T).astype(NPBF16)
    wq = np.ascontiguousarray(W_attn[:, 0:C][:, cols]).astype(NPBF16)
    wk = np.ascontiguousarray(W_attn[:, C : 2 * C][:, cols]).astype(NPBF16)
    wv = np.ascontiguousarray(W_attn[:, 2 * C : 3 * C][:, cols]).astype(NPBF16)
    wp = np.ascontiguousarray(W_proj[cols, :]).astype(NPBF16)
    p = np.arange(128)
    qb = np.empty((128, J), np.float32)
    kb = np.empty((128, J), np.float32)
    for jj in range(J):
        h = hg * HC + 2 * jj + p // 64
        d = p % 64
        qb[:, jj] = b_attn[h * D + d] + bQ[h, d]
        kb[:, jj] = b_attn[C + h * D + d] + bK[h, d]
    return {
        "xT": xT,
        "wq": wq,
        "wk": wk,
        "wv": wv,
        "wp": wp,
        "qb": qb,
        "kb": kb,
        "strip": strip_np,
    }


def kernel(x, W_attn, b_attn, bQ, bK, W_proj, b_proj):
    from concourse.bass_utils import run_bass_kernel_spmd

    x = np.asarray(x, np.float32)
    W_attn = np.asarray(W_attn, np.float32)
    b_attn = np.asarray(b_attn, np.float32)
    bQ = np.asarray(bQ, np.float32)
    bK = np.asarray(bK, np.float32)
    W_proj = np.asarray(W_proj, np.float32)
    b_proj = np.asarray(b_proj, np.float32)

    pcol = np.arange(128)[:, None]
    ccol = np.arange(512)[None, :]
    strip_np = (ccol >= pcol + 384).astype(NPBF16)

    in_maps = []
    for core in range(N_CORES):
        b_, hg = core // 2, core % 2
        in_maps.append(
            _prep_core_inputs(x, W_attn, b_attn, bQ, bK, W_proj, strip_np, b_, hg)
        )

    nc = _get_nc()
    res = run_bass_kernel_spmd(
        nc, in_maps, core_ids=list(range(N_CORES)), trace=TRACE
    )
    global LAST_RES
    LAST_RES = res

    # host-side combine: sum head-group partials, add b_proj and the folded
    # v-bias contribution (softmax rows sum to 1 => y += b_v exactly)
    bias = b_proj + b_attn[2 * C : 3 * C] @ W_proj
    out = np.empty((B, T, C), np.float32)
    for b_ in range(B):
        out[b_] = res.results[2 * b_]["out"] + res.results[2 * b_ + 1]["out"] + bias
    return out


if __name__ == "__main__":
    # quick shape smoke
    rng = np.random.default_rng(0)
    ins = {
        "x": rng.standard_normal((B, T, C), np.float32),
        "W_attn": rng.standard_normal((C, 3 * C), np.float32) * 0.02,
        "b_attn": rng.standard_normal((3 * C,), np.float32) * 0.02,
        "bQ": rng.standard_normal((NH, D), np.float32) * 0.1 + 0.5,
        "bK": rng.standard_normal((NH, D), np.float32) * 0.1 + 0.3,
        "W_proj": rng.standard_normal((C, C), np.float32) * 0.02,
        "b_proj": rng.standard_normal((C,), np.float32) * 0.02,
    }
    print(kernel(**ins).shape)
